# revision 51
# baseline (speedup 1.0000x reference)
"""Multi-head self-attention (B=2, N=2048, D=1024, 16 heads) on 8 TRN2
NeuronCores — tensor-parallel over heads (2 heads per core), row-parallel
output projection summed on the host.

Self-contained: takes the FULL inputs of reference.setup_inputs() and
returns the FULL [2, 2048, 1024] fp32 output.

Per-core device kernel (Bass/Tile, bf16 matmuls, fp32 accumulation):
    xT [1024, 4096]  (host-pretransposed activations, chunk-major DMA so
    the first QKV matmul starts ~13us in instead of ~21us)
    Q^T/K^T stacks [128(2 heads x 64), 4096];  V [tok, 2, 64] + ones col
    S^T chunks in PSUM -> one ACT exp per n-tile (scale folded, both
    heads' score matmuls in disjoint PE row groups run concurrently)
    -> P^T bf16;  AV emission runs 2 n-tiles behind scores/exp so the
    in-order PE queue never stalls on the ACT exp
    PV matmul with a ones column emits softmax denominators for free
    normalization: DVE reciprocal halves + one K=64 bf16 PE broadcast
    matmul building both heads' [128, 512] scale rows; the recip/R/mul/
    proj stages of unit u pop at fixed nt slots of unit u+1 (R far from
    the recips so it never blocks the PE queue)
    last 512-query unit split into two 256-query halves so its norm+
    proj chain overlaps the second half instead of the kernel tail;
    tail PSUM->SBUF copies go to the (then idle) ACT engine
    row-parallel proj partial [4096, 1024] bf16 -> DRAM (per-mt DMA),
    host sums the 8 partials in fp32 and adds the bias
"""

import sys
import types

sys.path.insert(0, "/opt/trn_rl_repo")

import numpy as np
import ml_dtypes

import concourse.mybir as mybir
import concourse.tile as tile
import concourse.bass as bass
from concourse.bass import ts
from concourse import bass_utils
from concourse.bass_utils import run_bass_kernel_spmd

# ─────────────────────────────────────────────────────────────────────
# Environment patches (this walrus build allows only ONE sem wait per
# instruction; Tile emits several — split them into single-wait nops).
# ─────────────────────────────────────────────────────────────────────


def _patched_drain_and_barrier(self, tick_clock, wait_clock):
    from concourse.tile import ScopedClock

    nc = self.nc
    drain_inst = nc.sync.drain()
    wait_clock.add_sem_waits(
        drain_inst.ins, ScopedClock({None: tick_clock.global_clock})
    )
    waits = list(drain_inst.ins.sync_info.on_wait)
    if len(waits) > 1:
        name2sem = {}
        for k, h in self.sems.allocated().items():
            nm = getattr(h, "name", None) or str(k)
            name2sem[nm] = h
        drain_inst.ins.sync_info = mybir.SyncInfo(
            on_wait=[waits[0]], on_update=[]
        )
        for w in waits[1:]:
            h = name2sem.get(w.ant_name)
            assert h is not None, (w.ant_name, list(name2sem))
            n = nc.sync.nop(nofuse=True)
            n.wait_op(h, w.wait_value, "sem-ge")
    nc.all_engine_barrier()
    popped = nc._tile_sem_poison_stack.pop()
    assert popped is self._sem_poison
    nc.clear_and_free_semaphores(list(self.sems.allocated().values()))
    nc.all_engine_barrier()


tile.TileContext._drain_and_barrier = _patched_drain_and_barrier
bass_utils.upload_artifacts = lambda tmpdir: tmpdir

_legalize_counter = [0]


def legalize_waits(nc):
    n_split = 0
    for f in nc.m.functions:
        for bb in f.blocks:
            insts = bb.instructions
            if not any(
                i.sync_info is not None and len(i.sync_info.on_wait) > 1
                for i in insts
            ):
                continue
            new_list = []
            for ins in insts:
                si = ins.sync_info
                if si is not None and len(si.on_wait) > 1:
                    waits = list(si.on_wait)
                    for w in waits[:-1]:
                        _legalize_counter[0] += 1
                        nop = mybir.InstNoOp(
                            name=f"lw_nop_{_legalize_counter[0]}",
                            ins=[], outs=[],
                        )
                        nop.engine = ins.engine
                        nop.sync_info = mybir.SyncInfo(
                            on_wait=[w], on_update=[]
                        )
                        new_list.append(nop)
                        n_split += 1
                    ins.sync_info = mybir.SyncInfo(
                        on_wait=[waits[-1]], on_update=list(si.on_update)
                    )
                new_list.append(ins)
            bb.instructions = new_list
    return n_split


# ─────────────────────────────────────────────────────────────────────
# Kernel build
# ─────────────────────────────────────────────────────────────────────

F32 = mybir.dt.float32
F16 = mybir.dt.float16
F32R = mybir.dt.float32r
BF16 = mybir.dt.bfloat16

DIM = 1024
HD = 64
B = 2
N = 2048
BN = B * N
SCALE = HD ** -0.5
N_CORES = 8
KT = DIM // 128
MC_B = N // 512  # 4
NT_B = N // 128  # 16


def _build_nc():
    mm_dt = BF16
    p_dt = BF16
    nc = bass.Bass("TRN2", target_bir_lowering=False, debug=False,
                   num_devices=N_CORES)
    xT = nc.dram_tensor("xT", [DIM, BN], mm_dt, kind="ExternalInput")
    # host pre-arranges qkv weights to [p, kt*m] so the DMA moves
    # contiguous 2KB partition lines instead of scattered 256B ones
    wq = nc.dram_tensor("wq", [128, DIM], mm_dt, kind="ExternalInput")
    wk = nc.dram_tensor("wk", [128, DIM], mm_dt, kind="ExternalInput")
    wv = nc.dram_tensor("wv", [128, DIM], mm_dt, kind="ExternalInput")
    wp = nc.dram_tensor("wp", [128, DIM], mm_dt, kind="ExternalInput")
    out = nc.dram_tensor("out", [BN, DIM], BF16, kind="ExternalOutput")
    out_t = out.ap().rearrange("(c p) d -> p c d", p=128)

    with tile.TileContext(nc) as tc:
        with (
            tc.tile_pool(name="xp", bufs=1) as xp,
            tc.tile_pool(name="wpool", bufs=1) as wpool,
            tc.tile_pool(name="qk", bufs=1) as qk,
            tc.tile_pool(name="pt", bufs=4) as ptp,
            tc.tile_pool(name="small", bufs=2) as sp,
            tc.tile_pool(name="osb", bufs=2) as osbp,
            tc.tile_pool(name="ostage", bufs=3) as osp,
            tc.tile_pool(name="ps_s", bufs=1, space="PSUM") as ps_s,
            tc.tile_pool(name="ps_big", bufs=2, space="PSUM") as ps_big,
            tc.tile_pool(name="ps_o", bufs=2, space="PSUM") as ps_o,
        ):
            # weights first (gpsimd queues) so QKV isn't stuck behind
            # the 8MB xT transfer; xT split per (k-tile, 512-token chunk)
            # ordered chunk-major so the first QKV matmul can start after
            # ~1MB instead of the full 8MB
            wq_s = wpool.tile([128, KT, 128], mm_dt, tag="wq")
            wk_s = wpool.tile([128, KT, 128], mm_dt, tag="wk")
            wv_s = wpool.tile([128, KT, 128], p_dt, tag="wv")
            for w_d, w_s in ((wq, wq_s), (wk, wk_s), (wv, wv_s)):
                nc.gpsimd.dma_start(
                    out=w_s[:],
                    in_=w_d.ap().rearrange("p (kt m) -> p kt m", kt=KT),
                )
            wp_s = wpool.tile([128, DIM], mm_dt, tag="wp")
            nc.gpsimd.dma_start(out=wp_s[:], in_=wp.ap())
            xT_s = xp.tile([128, KT, BN], mm_dt, tag="xT")
            xT_t = xT.ap().rearrange("(kt p) m -> p kt m", p=128)
            for mc in range(4):
                for kt in range(KT):
                    nc.sync.dma_start(
                        out=xT_s[:, kt, ts(mc, 512)],
                        in_=xT_t[:, kt, ts(mc, 512)],
                    )
            for kt in range(KT):
                nc.sync.dma_start(
                    out=xT_s[:, kt, N:BN],
                    in_=xT_t[:, kt, N:BN],
                )

            # ---- QKV projection work items ----
            QT_s = qk.tile([128, BN], mm_dt, tag="QT")
            KT_s = qk.tile([128, BN], mm_dt, tag="KT")
            V_s = qk.tile([128, 32, 2, 72], p_dt, tag="V")
            nc.vector.memset(V_s[:, :, :, 64], 1.0)

            def qkv_qt_kt(w_s, dst, mc):
                ps = ps_big.tile([128, 512], F32, tag="big", name="qk_ps")
                for kt in range(KT):
                    nc.tensor.matmul(
                        ps[:], w_s[:, kt, :], xT_s[:, kt, ts(mc, 512)],
                        start=(kt == 0), stop=(kt == KT - 1),
                    )
                nc.vector.tensor_copy(dst[:, ts(mc, 512)], ps[:])

            def qkv_v(mt):
                ps = ps_big.tile([128, 512], F32, tag="big", name="v_ps")
                for kt in range(KT):
                    nc.tensor.matmul(
                        ps[:, 0:128], xT_s[:, kt, ts(mt, 128)],
                        wv_s[:, kt, :],
                        start=(kt == 0), stop=(kt == KT - 1),
                    )
                nc.vector.tensor_copy(V_s[:, mt, 0, 0:64], ps[:, 0:64])
                nc.vector.tensor_copy(V_s[:, mt, 1, 0:64], ps[:, 64:128])

            def item_kt(mc):
                return lambda: qkv_qt_kt(wk_s, KT_s, mc)

            def item_qt(mc):
                return lambda: qkv_qt_kt(wq_s, QT_s, mc)

            def item_v(mt):
                return lambda: qkv_v(mt)

            # drip-feed schedule: QKV groups interleave with attention
            # chunks so the PE stream stays dense without bursts
            pre = [[[] for _ in range(16)] for _ in range(16)]
            pre[0][0] = [item_kt(0), item_qt(0), item_v(0)]
            for nt in range(15):
                pre[0][nt].append(item_v(nt + 1))
            pre[0][2].append(item_kt(1))
            pre[0][6].append(item_kt(2))
            pre[0][10].append(item_kt(3))
            for u in (1, 2, 3, 4, 5, 6, 7):
                pre[u - 1][8].append(item_qt(u))
            pre[1][0].append(item_kt(4))
            pre[1][2].append(item_kt(5))
            pre[2][0].append(item_kt(6))
            pre[2][2].append(item_kt(7))
            for i in range(8):
                pre[3][(i // 2) * 2 + 4].append(item_v(16 + i))
            for i in range(8):
                pre[4][(i // 2) * 2].append(item_v(24 + i))

            # ---- attention ----
            # ones64: K=64 stationary for the per-head normalization
            # broadcast (partition bases must be 32-aligned, so the den
            # rows live at {0, 32}); row 0 -> out cols 0-63 (head 0),
            # row 32 -> out cols 64-127 (head 1), rest zero
            ones64_b = sp.tile([64, 128], mm_dt, tag="ones64", bufs=1)
            nc.vector.memset(ones64_b[:], 0.0)
            nc.vector.memset(ones64_b[0:1, 0:64], 1.0)
            nc.vector.memset(ones64_b[32:33, 64:128], 1.0)

            def norm_and_proj_stages(q_off, w, den_s, o_list,
                                     tail=False):
                """9 closures emitted at nt slots of the NEXT unit so
                each small PE/DVE block finds its inputs ready."""
                state = {}

                def st_copies():
                    # pack both heads' O^T into one [128, w] SBUF
                    # tile (rows 0-63 head0, 64-127 head1) + den rows
                    o_sb = osbp.tile([128, 512], F32, tag="osb",
                                     name="o_sb")
                    for h in range(2):
                        if tail:
                            nc.scalar.copy(o_sb[ts(h, 64), 0:w],
                                           o_list[h][0:64, 0:w])
                        else:
                            nc.vector.tensor_copy(o_sb[ts(h, 64), 0:w],
                                                  o_list[h][0:64, 0:w])
                        nc.vector.tensor_copy(
                            den_s[32 * h:32 * h + 1, 0:w],
                            o_list[h][64:65, 0:w])
                    state["o_sb"] = o_sb

                def mk_recip(half):
                    def st_recip():
                        if "r" not in state:
                            state["r"] = sp.tile([64, 512], F32,
                                                 tag="rall",
                                                 name="r_all")
                        nc.vector.reciprocal(
                            state["r"][:, ts(half, w // 2)],
                            den_s[:, ts(half, w // 2)])
                    return st_recip

                def st_rmat():
                    rb = sp.tile([64, 512], mm_dt, tag="rb", name="r_bf")
                    nc.vector.tensor_copy(rb[:, 0:w], state["r"][:, 0:w])
                    R_ps = ps_big.tile([128, 512], F32, tag="big",
                                       name="R_ps")
                    nc.tensor.matmul(
                        R_ps[:, 0:w], ones64_b[:], rb[:, 0:w],
                        start=True, stop=True,
                    )
                    state["R"] = R_ps

                def st_mul():
                    AT = sp.tile([128, 512], mm_dt, tag="AT", name="AT_s")
                    nc.vector.tensor_mul(AT[:, 0:w], state["o_sb"][:, 0:w],
                                         state["R"][:, 0:w])
                    state["AT"] = AT

                def mk_proj(mt):
                    def st_proj():
                        if "out" not in state:
                            state["out"] = osp.tile(
                                [128, 4, DIM], BF16, tag="out",
                                name="out_stage",
                            )
                        for cc in range(2):
                            p_ps = ps_big.tile([128, 512], F32, tag="big")
                            nc.tensor.matmul(
                                p_ps[:], state["AT"][:, ts(mt, 128)],
                                wp_s[:, ts(cc, 512)],
                                start=True, stop=True,
                            )
                            if tail:
                                nc.scalar.copy(
                                    state["out"][:, mt, ts(cc, 512)],
                                    p_ps[:])
                            else:
                                nc.vector.tensor_copy(
                                    state["out"][:, mt, ts(cc, 512)],
                                    p_ps[:])
                        nc.sync.dma_start(
                            out=out_t[:, q_off // 128 + mt, :],
                            in_=state["out"][:, mt, :],
                        )
                    return st_proj

                return [st_copies, mk_recip(0), mk_recip(1), st_rmat,
                        st_mul] + [mk_proj(mt) for mt in range(w // 128)]

            pending = None
            av_q = []  # AV emission runs 2 nt behind scores/exp so the
            # in-order PE queue never stalls waiting on the ACT exp

            # last 512-query unit split into two 256-query halves so its
            # norm+proj chain overlaps the second half instead of
            # serializing into the kernel tail
            units = []
            for b in range(B):
                for mc in range(MC_B):
                    u = b * MC_B + mc
                    if u < 7:
                        units.append((b, u, u * 512, 512))
                    else:
                        units.append((b, u, u * 512, 256))
                        units.append((b, u, u * 512 + 256, 256))

            for b, unit, q_off, w in units:
                # den rows {0, 32}; memset (on the idle gpsimd
                # engine) so the unused rows can't feed inf/nan into
                # the bf16 broadcast matmul
                den_s = sp.tile([64, 512], F32, tag="den")
                nc.gpsimd.memset(den_s[:], 1.0)
                o_list = [ps_o.tile([128, 512], F32, tag="o",
                                    name=f"o_ps_{h}")
                          for h in range(2)]
                s_pair = None
                pt_pair = None
                for nt in range(NT_B):
                    for fn in pre[unit][nt] if q_off % 512 == 0 else []:
                        fn()
                    if nt in (2, 3, 4, 9, 10, 11, 12, 13, 14) \
                            and pending:
                        pending.pop(0)()
                    # both heads' score matmuls adjacent: disjoint PE
                    # row groups run concurrently; two nt share one
                    # 4-bank PSUM tile so the exp runs once per pair
                    # (halves the per-instruction ACT overhead)
                    if nt % 2 == 0:
                        s_pair = ps_s.tile([128, 4, 512], F32, tag="S")
                        pt_pair = ptp.tile([128, 4, 512], p_dt,
                                           tag="PT")
                    par = nt % 2
                    for h in range(2):
                        h_sl = ts(h, 64)
                        nc.tensor.matmul(
                            s_pair[:, 2 * par + h, 0:w],
                            KT_s[h_sl, b * N + nt * 128:
                                 b * N + (nt + 1) * 128],
                            QT_s[h_sl, q_off:q_off + w],
                            start=True, stop=True,
                        )
                    if par == 1:
                        nc.scalar.activation(
                            pt_pair[:, :, 0:w], s_pair[:, :, 0:w],
                            mybir.ActivationFunctionType.Exp,
                            scale=SCALE,
                        )

                    def mk_av(o_l, bb, ntt, pt, pp, ww):
                        def av():
                            for h in range(2):
                                nc.tensor.matmul(
                                    o_l[h][0:65, 0:ww],
                                    V_s[:, bb * NT_B + ntt, h, 0:65],
                                    pt[:, 2 * pp + h, 0:ww],
                                    start=(ntt == 0),
                                    stop=(ntt == NT_B - 1),
                                )
                        return av

                    av_q.append(mk_av(o_list, b, nt, pt_pair, par, w))
                    if par == 1:
                        while len(av_q) > 2:
                            av_q.pop(0)()
                # the unit's last two AVs drain at the next unit's
                # nt=0/1; the o_sb/den copies are the first pending
                # stage (popped at nt=2, after both AVs)
                pending = norm_and_proj_stages(
                    q_off, w, den_s, o_list,
                    tail=(q_off + w == BN))
            while av_q:
                av_q.pop(0)()
            for fn in pending:
                fn()
    legalize_waits(nc)
    return nc


_CACHE = {}


def _get_nc():
    if "nc" not in _CACHE:
        _CACHE["nc"] = _build_nc()
    return _CACHE["nc"]


def wpack_test(w):
    # [DIM, 128] -> [128p, KT*128] so each SBUF partition line is one
    # contiguous 2KB DMA read
    return np.ascontiguousarray(
        np.asarray(w, dtype=np.float32)
        .reshape(KT, 128, 128).transpose(1, 0, 2).reshape(128, DIM)
    ).astype(ml_dtypes.bfloat16)


def kernel(x, w_qkv, w_proj, b_proj):
    x = np.asarray(x, dtype=np.float32)
    w_qkv = np.asarray(w_qkv, dtype=np.float32)
    w_proj = np.asarray(w_proj, dtype=np.float32)
    b_proj = np.asarray(b_proj, dtype=np.float32)

    nc = _get_nc()
    bf = ml_dtypes.bfloat16

    xT = np.ascontiguousarray(x.reshape(BN, DIM).T).astype(bf)
    in_maps = []
    for c in range(N_CORES):
        sl = slice(128 * c, 128 * (c + 1))
        in_maps.append({
            "xT": xT,
            "wq": wpack_test(w_qkv[:, sl]),
            "wk": wpack_test(w_qkv[:, DIM + 128 * c:DIM + 128 * (c + 1)]),
            "wv": wpack_test(
                w_qkv[:, 2 * DIM + 128 * c:2 * DIM + 128 * (c + 1)]),
            "wp": np.ascontiguousarray(w_proj[sl, :]).astype(bf),
        })
    res = run_bass_kernel_spmd(nc, in_maps, list(range(N_CORES)),
                               trace=False)
    acc = res.results[0]["out"].astype(np.float32).copy()
    for c in range(1, N_CORES):
        acc += res.results[c]["out"]
    acc += b_proj[None, :]
    return acc.reshape(B, N, DIM)


# revision 52
# speedup vs baseline: 1.4146x; 1.4146x over previous
"""Multi-head self-attention (B=2, N=2048, D=1024, 16 heads) on 8 TRN2
NeuronCores — tensor-parallel over heads (2 heads per core), row-parallel
output projection summed on the host.

Self-contained: takes the FULL inputs of reference.setup_inputs() and
returns the FULL [2, 2048, 1024] fp32 output.

Per-core device kernel (Bass/Tile, bf16 matmuls, fp32 accumulation):
    xT [1024, 4096]  (host-pretransposed activations, chunk-major DMA so
    the first QKV matmul starts ~13us in instead of ~21us)
    Q^T/K^T stacks [128(2 heads x 64), 4096];  V [tok, 2, 64] + ones col
    S^T chunks in PSUM -> one ACT exp per n-tile (scale folded, both
    heads' score matmuls in disjoint PE row groups run concurrently)
    -> P^T bf16;  AV emission runs 2 n-tiles behind scores/exp so the
    in-order PE queue never stalls on the ACT exp
    PV matmul with a ones column emits softmax denominators for free
    normalization: DVE reciprocal halves + one K=64 bf16 PE broadcast
    matmul building both heads' [128, 512] scale rows; the recip/R/mul/
    proj stages of unit u pop at fixed nt slots of unit u+1 (R far from
    the recips so it never blocks the PE queue)
    last 512-query unit split into two 256-query halves so its norm+
    proj chain overlaps the second half instead of the kernel tail;
    tail PSUM->SBUF copies go to the (then idle) ACT engine
    row-parallel proj partial [4096, 1024] bf16 -> DRAM (per-mt DMA),
    host sums the 8 partials in fp32 and adds the bias
"""

import sys
import types

sys.path.insert(0, "/opt/trn_rl_repo")

import numpy as np
import ml_dtypes

import concourse.mybir as mybir
import concourse.tile as tile
import concourse.bass as bass
from concourse.bass import ts
from concourse import bass_utils
from concourse.bass_utils import run_bass_kernel_spmd

# ─────────────────────────────────────────────────────────────────────
# Environment patches (this walrus build allows only ONE sem wait per
# instruction; Tile emits several — split them into single-wait nops).
# ─────────────────────────────────────────────────────────────────────


def _patched_drain_and_barrier(self, tick_clock, wait_clock):
    from concourse.tile import ScopedClock

    nc = self.nc
    drain_inst = nc.sync.drain()
    wait_clock.add_sem_waits(
        drain_inst.ins, ScopedClock({None: tick_clock.global_clock})
    )
    waits = list(drain_inst.ins.sync_info.on_wait)
    if len(waits) > 1:
        name2sem = {}
        for k, h in self.sems.allocated().items():
            nm = getattr(h, "name", None) or str(k)
            name2sem[nm] = h
        drain_inst.ins.sync_info = mybir.SyncInfo(
            on_wait=[waits[0]], on_update=[]
        )
        for w in waits[1:]:
            h = name2sem.get(w.ant_name)
            assert h is not None, (w.ant_name, list(name2sem))
            n = nc.sync.nop(nofuse=True)
            n.wait_op(h, w.wait_value, "sem-ge")
    nc.all_engine_barrier()
    popped = nc._tile_sem_poison_stack.pop()
    assert popped is self._sem_poison
    nc.clear_and_free_semaphores(list(self.sems.allocated().values()))
    nc.all_engine_barrier()


tile.TileContext._drain_and_barrier = _patched_drain_and_barrier
bass_utils.upload_artifacts = lambda tmpdir: tmpdir

_legalize_counter = [0]


def legalize_waits(nc):
    n_split = 0
    for f in nc.m.functions:
        for bb in f.blocks:
            insts = bb.instructions
            if not any(
                i.sync_info is not None and len(i.sync_info.on_wait) > 1
                for i in insts
            ):
                continue
            new_list = []
            for ins in insts:
                si = ins.sync_info
                if si is not None and len(si.on_wait) > 1:
                    waits = list(si.on_wait)
                    for w in waits[:-1]:
                        _legalize_counter[0] += 1
                        nop = mybir.InstNoOp(
                            name=f"lw_nop_{_legalize_counter[0]}",
                            ins=[], outs=[],
                        )
                        nop.engine = ins.engine
                        nop.sync_info = mybir.SyncInfo(
                            on_wait=[w], on_update=[]
                        )
                        new_list.append(nop)
                        n_split += 1
                    ins.sync_info = mybir.SyncInfo(
                        on_wait=[waits[-1]], on_update=list(si.on_update)
                    )
                new_list.append(ins)
            bb.instructions = new_list
    return n_split


# ─────────────────────────────────────────────────────────────────────
# Kernel build
# ─────────────────────────────────────────────────────────────────────

F32 = mybir.dt.float32
F16 = mybir.dt.float16
F32R = mybir.dt.float32r
BF16 = mybir.dt.bfloat16

DIM = 1024
HD = 64
B = 2
N = 2048
BN = B * N
SCALE = HD ** -0.5
N_CORES = 8
KT = DIM // 128
MC_B = N // 512  # 4
NT_B = N // 128  # 16


def _build_nc():
    mm_dt = BF16
    p_dt = BF16
    nc = bass.Bass("TRN2", target_bir_lowering=False, debug=False,
                   num_devices=N_CORES)
    xT = nc.dram_tensor("xT", [DIM, BN], mm_dt, kind="ExternalInput")
    # host pre-arranges qkv weights to [p, kt*m] so the DMA moves
    # contiguous 2KB partition lines instead of scattered 256B ones
    wq = nc.dram_tensor("wq", [128, DIM], mm_dt, kind="ExternalInput")
    wk = nc.dram_tensor("wk", [128, DIM], mm_dt, kind="ExternalInput")
    wv = nc.dram_tensor("wv", [128, DIM], mm_dt, kind="ExternalInput")
    wp = nc.dram_tensor("wp", [128, DIM], mm_dt, kind="ExternalInput")
    out = nc.dram_tensor("out", [BN, DIM], BF16, kind="ExternalOutput")
    out_t = out.ap().rearrange("(c p) d -> p c d", p=128)

    with tile.TileContext(nc) as tc:
        with (
            tc.tile_pool(name="xp", bufs=1) as xp,
            tc.tile_pool(name="wpool", bufs=1) as wpool,
            tc.tile_pool(name="qk", bufs=1) as qk,
            tc.tile_pool(name="pt", bufs=8) as ptp,
            tc.tile_pool(name="small", bufs=2) as sp,
            tc.tile_pool(name="osb", bufs=2) as osbp,
            tc.tile_pool(name="ostage", bufs=3) as osp,
            tc.tile_pool(name="ps_s", bufs=2, space="PSUM") as ps_s,
            tc.tile_pool(name="ps_big", bufs=2, space="PSUM") as ps_big,
            tc.tile_pool(name="ps_o", bufs=2, space="PSUM") as ps_o,
        ):
            # weights first (gpsimd queues) so QKV isn't stuck behind
            # the 8MB xT transfer; xT split per (k-tile, 512-token chunk)
            # ordered chunk-major so the first QKV matmul can start after
            # ~1MB instead of the full 8MB
            wq_s = wpool.tile([128, KT, 128], mm_dt, tag="wq")
            wk_s = wpool.tile([128, KT, 128], mm_dt, tag="wk")
            wv_s = wpool.tile([128, KT, 128], p_dt, tag="wv")
            for w_d, w_s in ((wq, wq_s), (wk, wk_s), (wv, wv_s)):
                nc.gpsimd.dma_start(
                    out=w_s[:],
                    in_=w_d.ap().rearrange("p (kt m) -> p kt m", kt=KT),
                )
            wp_s = wpool.tile([128, DIM], mm_dt, tag="wp")
            nc.gpsimd.dma_start(out=wp_s[:], in_=wp.ap())
            xT_s = xp.tile([128, KT, BN], mm_dt, tag="xT")
            xT_t = xT.ap().rearrange("(kt p) m -> p kt m", p=128)
            for mc in range(4):
                for kt in range(KT):
                    nc.sync.dma_start(
                        out=xT_s[:, kt, ts(mc, 512)],
                        in_=xT_t[:, kt, ts(mc, 512)],
                    )
            for kt in range(KT):
                nc.sync.dma_start(
                    out=xT_s[:, kt, N:BN],
                    in_=xT_t[:, kt, N:BN],
                )

            # ---- QKV projection work items ----
            QT_s = qk.tile([128, BN], mm_dt, tag="QT")
            KT_s = qk.tile([128, BN], mm_dt, tag="KT")
            V_s = qk.tile([128, 32, 2, 72], p_dt, tag="V")
            nc.vector.memset(V_s[:, :, :, 64], 1.0)

            def qkv_qt_kt(w_s, dst, mc):
                ps = ps_big.tile([128, 512], F32, tag="big", name="qk_ps")
                for kt in range(KT):
                    nc.tensor.matmul(
                        ps[:], w_s[:, kt, :], xT_s[:, kt, ts(mc, 512)],
                        start=(kt == 0), stop=(kt == KT - 1),
                    )
                nc.vector.tensor_copy(dst[:, ts(mc, 512)], ps[:])

            def qkv_v(mt):
                ps = ps_big.tile([128, 512], F32, tag="big", name="v_ps")
                for kt in range(KT):
                    nc.tensor.matmul(
                        ps[:, 0:128], xT_s[:, kt, ts(mt, 128)],
                        wv_s[:, kt, :],
                        start=(kt == 0), stop=(kt == KT - 1),
                    )
                nc.vector.tensor_copy(V_s[:, mt, 0, 0:64], ps[:, 0:64])
                nc.vector.tensor_copy(V_s[:, mt, 1, 0:64], ps[:, 64:128])

            def item_kt(mc):
                return lambda: qkv_qt_kt(wk_s, KT_s, mc)

            def item_qt(mc):
                return lambda: qkv_qt_kt(wq_s, QT_s, mc)

            def item_v(mt):
                return lambda: qkv_v(mt)

            # drip-feed schedule: QKV groups interleave with attention
            # chunks so the PE stream stays dense without bursts
            pre = [[[] for _ in range(16)] for _ in range(16)]
            pre[0][0] = [item_kt(0), item_qt(0), item_v(0)]
            for nt in range(15):
                pre[0][nt].append(item_v(nt + 1))
            pre[0][2].append(item_kt(1))
            pre[0][6].append(item_kt(2))
            pre[0][10].append(item_kt(3))
            for u in (1, 2, 3, 4, 5, 6, 7):
                pre[u - 1][8].append(item_qt(u))
            pre[1][0].append(item_kt(4))
            pre[1][2].append(item_kt(5))
            pre[2][0].append(item_kt(6))
            pre[2][2].append(item_kt(7))
            for i in range(8):
                pre[3][(i // 2) * 2 + 4].append(item_v(16 + i))
            for i in range(8):
                pre[4][(i // 2) * 2].append(item_v(24 + i))

            # ---- attention ----
            # ones64: K=64 stationary for the per-head normalization
            # broadcast (partition bases must be 32-aligned, so the den
            # rows live at {0, 32}); row 0 -> out cols 0-63 (head 0),
            # row 32 -> out cols 64-127 (head 1), rest zero
            ones64_b = sp.tile([64, 128], mm_dt, tag="ones64", bufs=1)
            nc.vector.memset(ones64_b[:], 0.0)
            nc.vector.memset(ones64_b[0:1, 0:64], 1.0)
            nc.vector.memset(ones64_b[32:33, 64:128], 1.0)

            def norm_and_proj_stages(q_off, w, den_s, o_list,
                                     tail=False):
                """9 closures emitted at nt slots of the NEXT unit so
                each small PE/DVE block finds its inputs ready."""
                state = {}

                def st_copies():
                    # pack both heads' O^T into one [128, w] SBUF
                    # tile (rows 0-63 head0, 64-127 head1) + den rows
                    o_sb = osbp.tile([128, 512], F32, tag="osb",
                                     name="o_sb")
                    for h in range(2):
                        if tail:
                            nc.scalar.copy(o_sb[ts(h, 64), 0:w],
                                           o_list[h][0:64, 0:w])
                        else:
                            nc.vector.tensor_copy(o_sb[ts(h, 64), 0:w],
                                                  o_list[h][0:64, 0:w])
                        nc.vector.tensor_copy(
                            den_s[32 * h:32 * h + 1, 0:w],
                            o_list[h][64:65, 0:w])
                    state["o_sb"] = o_sb

                def mk_recip(half):
                    def st_recip():
                        if "r" not in state:
                            state["r"] = sp.tile([64, 512], F32,
                                                 tag="rall",
                                                 name="r_all")
                        nc.vector.reciprocal(
                            state["r"][:, ts(half, w // 2)],
                            den_s[:, ts(half, w // 2)])
                    return st_recip

                def st_rmat():
                    rb = sp.tile([64, 512], mm_dt, tag="rb", name="r_bf")
                    nc.vector.tensor_copy(rb[:, 0:w], state["r"][:, 0:w])
                    R_ps = ps_big.tile([128, 512], F32, tag="big",
                                       name="R_ps")
                    nc.tensor.matmul(
                        R_ps[:, 0:w], ones64_b[:], rb[:, 0:w],
                        start=True, stop=True,
                    )
                    state["R"] = R_ps

                def st_mul():
                    AT = sp.tile([128, 512], mm_dt, tag="AT", name="AT_s")
                    nc.vector.tensor_mul(AT[:, 0:w], state["o_sb"][:, 0:w],
                                         state["R"][:, 0:w])
                    state["AT"] = AT

                def mk_proj(mt):
                    def st_proj():
                        if "out" not in state:
                            state["out"] = osp.tile(
                                [128, 4, DIM], BF16, tag="out",
                                name="out_stage",
                            )
                        for cc in range(2):
                            p_ps = ps_big.tile([128, 512], F32, tag="big")
                            nc.tensor.matmul(
                                p_ps[:], state["AT"][:, ts(mt, 128)],
                                wp_s[:, ts(cc, 512)],
                                start=True, stop=True,
                            )
                            if tail:
                                nc.scalar.copy(
                                    state["out"][:, mt, ts(cc, 512)],
                                    p_ps[:])
                            else:
                                nc.vector.tensor_copy(
                                    state["out"][:, mt, ts(cc, 512)],
                                    p_ps[:])
                        nc.sync.dma_start(
                            out=out_t[:, q_off // 128 + mt, :],
                            in_=state["out"][:, mt, :],
                        )
                    return st_proj

                return [st_copies, mk_recip(0), mk_recip(1), st_rmat,
                        st_mul] + [mk_proj(mt) for mt in range(w // 128)]

            pending = None
            av_q = []  # AV emission runs 2 nt behind scores/exp so the
            # in-order PE queue never stalls waiting on the ACT exp

            # last 512-query unit split into two 256-query halves so its
            # norm+proj chain overlaps the second half instead of
            # serializing into the kernel tail
            units = []
            for b in range(B):
                for mc in range(MC_B):
                    u = b * MC_B + mc
                    if u < 7:
                        units.append((b, u, u * 512, 512))
                    else:
                        units.append((b, u, u * 512, 256))
                        units.append((b, u, u * 512 + 256, 256))

            for b, unit, q_off, w in units:
                # den rows {0, 32}; memset (on the idle gpsimd
                # engine) so the unused rows can't feed inf/nan into
                # the bf16 broadcast matmul
                den_s = sp.tile([64, 512], F32, tag="den")
                nc.gpsimd.memset(den_s[:], 1.0)
                o_list = [ps_o.tile([128, 512], F32, tag="o",
                                    name=f"o_ps_{h}")
                          for h in range(2)]
                for nt in range(NT_B):
                    for fn in pre[unit][nt] if q_off % 512 == 0 else []:
                        fn()
                    if nt in (2, 3, 4, 9, 10, 11, 12, 13, 14) \
                            and pending:
                        pending.pop(0)()
                    # both heads' score matmuls adjacent: disjoint PE
                    # row groups run concurrently
                    s_ps = ps_s.tile([128, 2, 512], F32, tag="S")
                    for h in range(2):
                        h_sl = ts(h, 64)
                        nc.tensor.matmul(
                            s_ps[:, h, 0:w],
                            KT_s[h_sl, b * N + nt * 128:
                                 b * N + (nt + 1) * 128],
                            QT_s[h_sl, q_off:q_off + w],
                            start=True, stop=True,
                        )
                    PT_s = ptp.tile([128, 2, 512], p_dt, tag="PT")
                    nc.scalar.activation(
                        PT_s[:, :, 0:w], s_ps[:, :, 0:w],
                        mybir.ActivationFunctionType.Exp,
                        scale=SCALE,
                    )

                    def mk_av(o_l, bb, ntt, pt, ww):
                        def av():
                            for h in range(2):
                                nc.tensor.matmul(
                                    o_l[h][0:65, 0:ww],
                                    V_s[:, bb * NT_B + ntt, h, 0:65],
                                    pt[:, h, 0:ww],
                                    start=(ntt == 0),
                                    stop=(ntt == NT_B - 1),
                                )
                        return av

                    av_q.append(mk_av(o_list, b, nt, PT_s, w))
                    if len(av_q) > 2:
                        av_q.pop(0)()
                # the unit's last two AVs drain at the next unit's
                # nt=0/1; the o_sb/den copies are the first pending
                # stage (popped at nt=2, after both AVs)
                pending = norm_and_proj_stages(
                    q_off, w, den_s, o_list,
                    tail=(q_off + w == BN))
            while av_q:
                av_q.pop(0)()
            for fn in pending:
                fn()
    legalize_waits(nc)
    return nc


_CACHE = {}


def _get_nc():
    if "nc" not in _CACHE:
        _CACHE["nc"] = _build_nc()
    return _CACHE["nc"]


def wpack_test(w):
    # [DIM, 128] -> [128p, KT*128] so each SBUF partition line is one
    # contiguous 2KB DMA read
    return np.ascontiguousarray(
        np.asarray(w, dtype=np.float32)
        .reshape(KT, 128, 128).transpose(1, 0, 2).reshape(128, DIM)
    ).astype(ml_dtypes.bfloat16)


def kernel(x, w_qkv, w_proj, b_proj):
    x = np.asarray(x, dtype=np.float32)
    w_qkv = np.asarray(w_qkv, dtype=np.float32)
    w_proj = np.asarray(w_proj, dtype=np.float32)
    b_proj = np.asarray(b_proj, dtype=np.float32)

    nc = _get_nc()
    bf = ml_dtypes.bfloat16

    xT = np.ascontiguousarray(x.reshape(BN, DIM).T).astype(bf)
    in_maps = []
    for c in range(N_CORES):
        sl = slice(128 * c, 128 * (c + 1))
        in_maps.append({
            "xT": xT,
            "wq": wpack_test(w_qkv[:, sl]),
            "wk": wpack_test(w_qkv[:, DIM + 128 * c:DIM + 128 * (c + 1)]),
            "wv": wpack_test(
                w_qkv[:, 2 * DIM + 128 * c:2 * DIM + 128 * (c + 1)]),
            "wp": np.ascontiguousarray(w_proj[sl, :]).astype(bf),
        })
    res = run_bass_kernel_spmd(nc, in_maps, list(range(N_CORES)),
                               trace=False)
    acc = res.results[0]["out"].astype(np.float32).copy()
    for c in range(1, N_CORES):
        acc += res.results[c]["out"]
    acc += b_proj[None, :]
    return acc.reshape(B, N, DIM)
